# revision 1
# baseline (speedup 1.0000x reference)
# GATv2 encoder (3x GATv2Conv, H=1) on 8 Trainium2 NeuronCores.
#
# Sharding: nodes partitioned by dst across 8 cores (graph parallel).
# Edge work per core is organized as 98 "bins" of <=128 dst nodes each
# (host-side bin-packing balances edge counts); a bin's edges are grouped
# by source-table chunk (4 chunks of 25088 rows so indices fit int16) and
# padded to 128-edge tiles.  Per bin: batched dma_gather of source rows
# and xr rows, leaky-relu attention logits, exp, and per-tile one-hot
# matmuls on the TensorEngine that accumulate softmax numerator and
# denominator in PSUM.  Host gathers the per-core dense outputs between
# the three launches (all-gather of the xl tables).
import os
import sys
import math
import functools
import numpy as np

for _p in ("/opt/trn_rl_repo",):
    if _p not in sys.path and os.path.isdir(_p):
        sys.path.insert(0, _p)

import concourse.bass as bass
import concourse.mybir as mybir
import concourse.tile as tile
from concourse import bacc

F32 = mybir.dt.float32
BF16 = mybir.dt.bfloat16
I16 = mybir.dt.int16
I32 = mybir.dt.int32
AF = mybir.ActivationFunctionType
ALU = mybir.AluOpType

# Problem constants (hardcoded per contract)
N = 100_000
E = 1_600_000
IN, HID, OUT, H = 256, 128, 64, 1
SLOPE = 0.2
NCORES = 8
P = 128
EPS = 1e-30
NCHUNK = 4          # source-table chunks (rows per chunk must fit int16)


class Cfg:
    """Geometry, parameterized so small test instances can be built."""

    def __init__(self, n=N, e=E, fin=IN, hid=HID, out=OUT, ncores=NCORES):
        self.n, self.e, self.fin, self.hid, self.out = n, e, fin, hid, out
        self.ncores = ncores
        assert n % ncores == 0
        self.nl = n // ncores                  # dst nodes per core
        self.nbins = math.ceil(self.nl / P)    # bins per core
        self.nlp = self.nbins * P              # padded local nodes
        self.ntab = self.nlp * ncores          # rows in gathered tables
        self.nchunk = min(NCHUNK, ncores)
        assert self.ntab % self.nchunk == 0
        self.chrows = self.ntab // self.nchunk  # rows per source chunk
        assert self.chrows < 32768, "chunk rows must fit int16"
        assert fin % P == 0
        self.kt = fin // P                     # K-tiles for dense1


# ----------------------------------------------------------------------------
# Host-side graph preprocessing
# ----------------------------------------------------------------------------

def prep_graph(cfg: Cfg, edge_index: np.ndarray):
    """Bin-pack dsts, group edges by (bin, src chunk), build index arrays."""
    n, ncores, nl, nbins, nlp = cfg.n, cfg.ncores, cfg.nl, cfg.nbins, cfg.nlp
    nck, chrows = cfg.nchunk, cfg.chrows
    src = np.concatenate([edge_index[0], np.arange(n, dtype=np.int64)])
    dst = np.concatenate([edge_index[1], np.arange(n, dtype=np.int64)])

    # --- per-core bin-packing of dst nodes ---------------------------------
    slot_global = np.full(n, -1, dtype=np.int64)  # node -> row in table space
    deg_all = np.bincount(dst, minlength=n)
    import heapq
    for c in range(ncores):
        lo, hi = c * nl, (c + 1) * nl
        deg = deg_all[lo:hi]
        order = np.argsort(-deg, kind="stable")
        heap = [(0, 0, b) for b in range(nbins)]
        heapq.heapify(heap)
        stash = []
        for node in order:
            d = int(deg[node])
            while True:
                s, cnt, b = heapq.heappop(heap)
                if cnt < P:
                    break
                stash.append((s, cnt, b))
            slot_global[lo + node] = c * nlp + b * P + cnt
            heapq.heappush(heap, (s + d, cnt + 1, b))
            for it in stash:
                heapq.heappush(heap, it)
            stash.clear()

    # --- group edges by (core, bin, chunk) ---------------------------------
    sslot = slot_global[src]
    dslot = slot_global[dst]
    chunk = sslot // chrows
    binid = dslot // P                    # global bin id = core*nbins + bin
    key = binid * nck + chunk
    order = np.argsort(key, kind="stable")
    s_o, d_o, k_o = sslot[order], dslot[order], key[order]
    nkeys = ncores * nbins * nck
    cnts = np.bincount(k_o, minlength=nkeys).reshape(ncores, nbins, nck)
    offs = np.concatenate([[0], np.cumsum(cnts.reshape(-1))])

    # uniform-across-cores tiles per (bin, chunk)
    tbo = np.maximum(np.ceil(cnts / P).astype(np.int64).max(axis=0),
                     (cnts.max(axis=0) > 0))  # [nbins, nck]
    tbin = tbo.sum(axis=1)                 # [nbins] tiles per bin
    nslots = int(tbin.sum()) * P           # edge slots per core

    # --- per-core index arrays ---------------------------------------------
    # gidx16: wrapped-16 int16 chunk-local src indices, [128, nslots//16]
    # ridx16: wrapped-16 int16 local dst rows,          [128, nslots//16]
    # dstcol: lane-major one-hot columns (f32),         [128, nslots//128]
    gidx16 = np.zeros((ncores, 128, nslots // 16), np.int16)
    ridx16 = np.zeros((ncores, 128, nslots // 16), np.int16)
    dstcol = np.full((ncores, 128, nslots // 128), 200.0, np.float32)
    for c in range(ncores):
        pos = 0  # slot position within the core's stream
        for b in range(nbins):
            rbase = b * P
            for o in range(nck):
                kk = int(cnts[c, b, o])
                so = offs[(c * nbins + b) * nck + o]
                slots = int(tbo[b, o]) * P
                if slots == 0:
                    continue
                j = np.arange(kk)
                jp = pos + j
                g = np.zeros(slots, np.int16)
                g[j] = (s_o[so:so + kk] - o * chrows).astype(np.int16)
                r = np.full(slots, c * nlp + rbase, np.int64)
                r[j] = d_o[so:so + kk]
                r -= c * nlp
                jj = pos + np.arange(slots)
                gidx16[c, jj % 16, jj // 16] = g
                ridx16[c, jj % 16, jj // 16] = r.astype(np.int16)
                dstcol[c, jp % 128, jp // 128] = (
                    d_o[so:so + kk] - (c * nlp + rbase)).astype(np.float32)
                pos += slots
        assert pos == nslots
        # the Q7 gather ucode reads indices from its own 16-partition group:
        # replicate the wrapped-16 data across all 8 groups
        gidx16[c] = np.tile(gidx16[c, :16], (8, 1))
        ridx16[c] = np.tile(ridx16[c, :16], (8, 1))

    # node permutation per core: slot s -> original node (or -1)
    perm = np.full((ncores, nlp), -1, dtype=np.int64)
    nodes = np.where(slot_global >= 0)[0]
    perm.reshape(-1)[slot_global[nodes]] = nodes

    return dict(
        tbo=tbo, tbin=tbin, nslots=nslots, slot_global=slot_global, perm=perm,
        gidx16=gidx16, ridx16=ridx16, dstcol=dstcol,
    )


# ----------------------------------------------------------------------------
# Device program builders (single SPMD program, data differs per core)
# ----------------------------------------------------------------------------

def _new_nc(cfg, nq=1):
    return bacc.Bacc("TRN2", target_bir_lowering=False, debug=False,
                     enable_asserts=False, num_devices=cfg.ncores,
                     num_swdge_queues=nq)


def build_dense1(cfg: Cfg, dt=F32):
    """xT [fin, nlp] -> XL1 [nlp, hid], XR1 [nlp, hid]."""
    nc = _new_nc(cfg)
    fin, hid, nlp, kt = cfg.fin, cfg.hid, cfg.nlp, cfg.kt
    xT = nc.dram_tensor("xT", [fin, nlp], F32, kind="ExternalInput")
    wl = nc.dram_tensor("wl", [fin, hid], F32, kind="ExternalInput")
    wr = nc.dram_tensor("wr", [fin, hid], F32, kind="ExternalInput")
    blB = nc.dram_tensor("blB", [P, hid], F32, kind="ExternalInput")
    brB = nc.dram_tensor("brB", [P, hid], F32, kind="ExternalInput")
    XL = nc.dram_tensor("XL1", [nlp, hid], dt, kind="ExternalOutput")
    XR = nc.dram_tensor("XR1", [nlp, hid], dt, kind="ExternalOutput")

    mtiles = nlp // P
    with tile.TileContext(nc) as tc:
        with tc.tile_pool(name="const", bufs=1) as cp, \
             tc.tile_pool(name="work", bufs=4) as wp, \
             tc.tile_pool(name="psum", bufs=4, space="PSUM") as pp:
            xk = cp.tile([P, kt, nlp], F32)
            nc.sync.dma_start(xk[:], xT[:].rearrange("(k p) n -> p k n", p=P))
            wl_sb = cp.tile([P, kt, hid], F32)
            nc.sync.dma_start(wl_sb[:], wl[:].rearrange("(k p) h -> p k h", p=P))
            wr_sb = cp.tile([P, kt, hid], F32)
            nc.sync.dma_start(wr_sb[:], wr[:].rearrange("(k p) h -> p k h", p=P))
            blB_sb = cp.tile([P, hid], F32)
            nc.sync.dma_start(blB_sb[:], blB[:])
            brB_sb = cp.tile([P, hid], F32)
            nc.sync.dma_start(brB_sb[:], brB[:])

            for m in range(mtiles):
                ms = slice(m * P, (m + 1) * P)
                psl = pp.tile([P, hid], F32, tag="psl")
                psr = pp.tile([P, hid], F32, tag="psr")
                for k in range(kt):
                    nc.tensor.matmul(psl[:], lhsT=xk[:, k, ms], rhs=wl_sb[:, k, :],
                                     start=(k == 0), stop=(k == kt - 1))
                for k in range(kt):
                    nc.tensor.matmul(psr[:], lhsT=xk[:, k, ms], rhs=wr_sb[:, k, :],
                                     start=(k == 0), stop=(k == kt - 1))
                ol = wp.tile([P, hid], dt, tag="ol")
                nc.vector.tensor_tensor(out=ol[:], in0=psl[:], in1=blB_sb[:], op=ALU.add)
                orr = wp.tile([P, hid], dt, tag="orr")
                nc.vector.tensor_tensor(out=orr[:], in0=psr[:], in1=brB_sb[:], op=ALU.add)
                nc.sync.dma_start(XL[ms, :], ol[:])
                nc.sync.dma_start(XR[ms, :], orr[:])
    nc.compile()
    return nc


def _edge_phase(nc, tc, cfg, pr, pools, tabs, consts, n_lay, finalize, dt):
    """Shared edge-pipeline over bins.

    Gathered rows are 128 wide: n_lay layer blocks of feat=128//n_lay cols.
    finalize(b, psums): consume accumulated PSUM tiles per bin.
    """
    from concourse import library_config
    nc.gpsimd.load_library(library_config.mlp)
    cp, gp, wp, pp = pools
    XLchunks, XR, GIDX, RIDX, DCOL = tabs
    iotaRep_sb, attB_sb, ones_sb = consts
    tbo, tbin = pr["tbo"], pr["tbin"]
    nslots = pr["nslots"]
    nck, chrows = cfg.nchunk, cfg.chrows
    feat = P // n_lay
    Tmax = int(tbin.max())
    NQ = nc.num_swdge_queues
    qn = 0

    # whole-launch index arrays resident in SBUF
    gix = cp.tile([P, nslots // 16], I16)
    nc.sync.dma_start(gix[:], GIDX[:])
    rix = cp.tile([P, nslots // 16], I16)
    nc.sync.dma_start(rix[:], RIDX[:])
    dcl = cp.tile([P, nslots // 128, 1], dt)
    nc.sync.dma_start(dcl[:], DCOL[:])

    pos = 0
    for b in range(cfg.nbins):
        Tb = int(tbin[b])
        G = gp.tile([P, Tmax, P], dt, tag="G", name="G")
        R = gp.tile([P, Tmax, P], dt, tag="R", name="R")
        to = 0
        for o in range(nck):
            tt = int(tbo[b, o])
            if tt == 0:
                continue
            nidx = tt * P
            col = (pos + to * P) // 16
            nc.gpsimd.dma_gather(
                out_ap=G[:, to:to + tt, :],
                in_ap=XLchunks[o][:],
                idxs_ap=gix[:, col:col + nidx // 16],
                num_idxs=nidx, num_idxs_reg=nidx, elem_size=P,
                single_packet=(nidx <= 1024), queue_num=qn % NQ)
            qn += 1
            to += tt
        assert to == Tb
        nc.gpsimd.dma_gather(
            out_ap=R[:, 0:Tb, :], in_ap=XR[:],
            idxs_ap=rix[:, pos // 16:(pos + Tb * P) // 16],
            num_idxs=Tb * P, num_idxs_reg=Tb * P, elem_size=P,
            single_packet=(Tb * P <= 1024), queue_num=qn % NQ)
        qn += 1
        dcol = dcl[:, pos // P:(pos + Tb * P) // P, :]   # [P, Tb, 1]

        # z = xl[src] + xr[dst]
        nc.vector.tensor_tensor(out=R[:, 0:Tb, :], in0=R[:, 0:Tb, :],
                                in1=G[:, 0:Tb, :], op=ALU.add)
        # leaky_relu(z) = max(0.2*z, z), then * att
        U = wp.tile([P, Tmax, P], dt, tag="U", name="U")
        nc.vector.scalar_tensor_tensor(
            out=U[:, 0:Tb, :], in0=R[:, 0:Tb, :], scalar=SLOPE,
            in1=R[:, 0:Tb, :], op0=ALU.mult, op1=ALU.max)
        nc.vector.tensor_tensor(out=U[:, 0:Tb, :], in0=U[:, 0:Tb, :],
                                in1=attB_sb[:, 0:Tb, :], op=ALU.mult)
        e = wp.tile([P, Tmax, n_lay, 1], F32, tag="e", name="e")
        nc.vector.tensor_reduce(
            out=e[:, 0:Tb, :, :],
            in_=U[:, 0:Tb, :].rearrange("p t (l f) -> p t l f", l=n_lay),
            axis=mybir.AxisListType.X, op=ALU.add)
        ee = wp.tile([P, Tmax, n_lay, 1], dt, tag="ee", name="ee")
        nc.scalar.activation(out=ee[:, 0:Tb, :, :], in_=e[:, 0:Tb, :, :],
                             func=AF.Exp)

        # batched one-hot * ee build (per-bin TTs, not per-tile tensor_scalar)
        O = wp.tile([P, Tmax, 1, P], dt, tag="O", name="O")
        nc.vector.tensor_tensor(
            out=O[:, 0:Tb, 0, :],
            in0=dcol.to_broadcast([P, Tb, P]),
            in1=iotaRep_sb[:, 0:Tb, :], op=ALU.is_equal)
        A = wp.tile([P, Tmax, n_lay, P], dt, tag="A", name="A")
        nc.vector.tensor_tensor(
            out=A[:, 0:Tb, :, :],
            in0=O[:, 0:Tb, :, :].to_broadcast([P, Tb, n_lay, P]),
            in1=ee[:, 0:Tb, :, :].to_broadcast([P, Tb, n_lay, P]),
            op=ALU.mult)

        # num and den must live in different PSUM banks: start=True zeroes
        # the whole 2KB zero-region, so interleaved groups can't share one.
        pnum = [pp.tile([P, feat], F32, tag=f"pn{l}", name=f"pn{l}")
                for l in range(n_lay)]
        pden = [pp.tile([P, 1], F32, tag=f"pd{l}", name=f"pd{l}")
                for l in range(n_lay)]
        for t in range(Tb):
            for l in range(n_lay):
                nc.tensor.matmul(
                    pnum[l][:], lhsT=A[:, t, l, :],
                    rhs=G[:, t, l * feat:(l + 1) * feat],
                    start=(t == 0), stop=(t == Tb - 1))
                nc.tensor.matmul(
                    pden[l][:], lhsT=A[:, t, l, :], rhs=ones_sb[:],
                    start=(t == 0), stop=(t == Tb - 1))
        finalize(b, list(zip(pnum, pden)))
        pos += Tb * P


def _load_consts(nc, cp, names_shapes):
    out = []
    for name, shape, dt in names_shapes:
        dram = nc.dram_tensor(name, shape, dt, kind="ExternalInput")
        sb = cp.tile(shape, dt, name=name + "_sb")
        nc.sync.dma_start(sb[:], dram[:])
        out.append(sb)
    return out


def build_edge1(cfg: Cfg, pr, dt=F32, nq=1):
    """Edge phase of layer 1 + dense transforms of layers 2/3."""
    nc = _new_nc(cfg, nq)
    hid, out, nlp, nbins = cfg.hid, cfg.out, cfg.nlp, cfg.nbins
    nslots = pr["nslots"]
    Tmax = int(pr["tbin"].max())
    XLchunks = [nc.dram_tensor(f"XL1c{o}", [cfg.chrows, hid], dt,
                               kind="ExternalInput") for o in range(cfg.nchunk)]
    XR = nc.dram_tensor("XR1", [nlp, hid], dt, kind="ExternalInput")
    GIDX = nc.dram_tensor("gidx", [P, nslots // 16], I16, kind="ExternalInput")
    RIDX = nc.dram_tensor("ridx", [P, nslots // 16], I16, kind="ExternalInput")
    DCOL = nc.dram_tensor("dcol", [P, nslots // 128, 1], dt, kind="ExternalInput")
    XL23 = nc.dram_tensor("XL23", [nlp, P], dt, kind="ExternalOutput")
    XR23 = nc.dram_tensor("XR23", [nlp, P], dt, kind="ExternalOutput")

    with tile.TileContext(nc) as tc:
        with tc.tile_pool(name="const", bufs=1) as cp, \
             tc.tile_pool(name="gath", bufs=2) as gp, \
             tc.tile_pool(name="work", bufs=2) as wp, \
             tc.tile_pool(name="psum", bufs=2, space="PSUM") as pp, \
             tc.tile_pool(name="psfin", bufs=1, space="PSUM") as pf:
            (iotaRep_sb, attB_sb, b1B_sb, ident_sb, w23l_sb, w23r_sb,
             b23l_sb, b23r_sb) = _load_consts(nc, cp, [
                 ("iotaRep", [P, Tmax, P], dt),
                 ("attB", [P, Tmax, hid], dt),
                 ("b1B", [P, hid], F32),
                 ("identB", [P, P], F32),
                 ("w23l", [hid, P], F32),
                 ("w23r", [hid, P], F32),
                 ("b23lB", [P, P], F32),
                 ("b23rB", [P, P], F32)])
            ones_sb = cp.tile([P, 1], dt)
            nc.vector.memset(ones_sb[:], 1.0)

            def finalize(b, psums):
                pn, pd = psums[0]
                ms = slice(b * P, (b + 1) * P)
                d = wp.tile([P, 1], F32, tag="d", name="d")
                nc.vector.tensor_scalar_add(d[:], pd[:], EPS)
                r = wp.tile([P, 1], F32, tag="r", name="r")
                nc.vector.reciprocal(r[:], d[:])
                h = wp.tile([P, hid], F32, tag="h", name="h")
                nc.vector.tensor_scalar(out=h[:], in0=pn[:], scalar1=r[:],
                                        scalar2=None, op0=ALU.mult)
                nc.vector.tensor_tensor(out=h[:], in0=h[:], in1=b1B_sb[:], op=ALU.add)
                nc.scalar.activation(out=h[:], in_=h[:], func=AF.Relu)
                pst = pf.tile([P, P], F32, tag="pst", name="pst")
                nc.tensor.transpose(out=pst[:], in_=h[:], identity=ident_sb[:])
                hT = wp.tile([P, P], F32, tag="hT", name="hT")
                nc.vector.tensor_copy(hT[:], pst[:])
                psl = pf.tile([P, P], F32, tag="psl", name="psl")
                nc.tensor.matmul(psl[:], lhsT=hT[:, 0:hid], rhs=w23l_sb[:],
                                 start=True, stop=True)
                psr = pf.tile([P, P], F32, tag="psr", name="psr")
                nc.tensor.matmul(psr[:], lhsT=hT[:, 0:hid], rhs=w23r_sb[:],
                                 start=True, stop=True)
                ol = wp.tile([P, P], dt, tag="ol", name="ol")
                nc.vector.tensor_tensor(out=ol[:], in0=psl[:], in1=b23l_sb[:], op=ALU.add)
                orr = wp.tile([P, P], dt, tag="orr", name="orr")
                nc.vector.tensor_tensor(out=orr[:], in0=psr[:], in1=b23r_sb[:], op=ALU.add)
                nc.sync.dma_start(XL23[ms, :], ol[:])
                nc.sync.dma_start(XR23[ms, :], orr[:])

            _edge_phase(nc, tc, cfg, pr, (cp, gp, wp, pp),
                        (XLchunks, XR, GIDX, RIDX, DCOL),
                        (iotaRep_sb, attB_sb, ones_sb), 1, finalize, dt)
    nc.compile()
    return nc


def build_edge23(cfg: Cfg, pr, dt=F32, nq=1):
    """Edge phases of layers 2 and 3 (shared gather)."""
    nc = _new_nc(cfg, nq)
    out, nlp, nbins = cfg.out, cfg.nlp, cfg.nbins
    nslots = pr["nslots"]
    Tmax = int(pr["tbin"].max())
    XLchunks = [nc.dram_tensor(f"XL23c{o}", [cfg.chrows, P], dt,
                               kind="ExternalInput") for o in range(cfg.nchunk)]
    XR = nc.dram_tensor("XR23", [nlp, P], dt, kind="ExternalInput")
    GIDX = nc.dram_tensor("gidx", [P, nslots // 16], I16, kind="ExternalInput")
    RIDX = nc.dram_tensor("ridx", [P, nslots // 16], I16, kind="ExternalInput")
    DCOL = nc.dram_tensor("dcol", [P, nslots // 128, 1], dt, kind="ExternalInput")
    MU = nc.dram_tensor("MU", [nlp, out], F32, kind="ExternalOutput")
    LV = nc.dram_tensor("LV", [nlp, out], F32, kind="ExternalOutput")

    with tile.TileContext(nc) as tc:
        with tc.tile_pool(name="const", bufs=1) as cp, \
             tc.tile_pool(name="gath", bufs=2) as gp, \
             tc.tile_pool(name="work", bufs=2) as wp, \
             tc.tile_pool(name="psum", bufs=2, space="PSUM") as pp:
            iotaRep_sb, attB_sb, bmu_sb, blv_sb = _load_consts(nc, cp, [
                ("iotaRep", [P, Tmax, P], dt),
                ("attB", [P, Tmax, P], dt),
                ("bmuB", [P, out], F32),
                ("blvB", [P, out], F32)])
            ones_sb = cp.tile([P, 1], dt)
            nc.vector.memset(ones_sb[:], 1.0)

            def finalize(b, psums):
                ms = slice(b * P, (b + 1) * P)
                for (pn, pd), bias, dest, tg in ((psums[0], bmu_sb, MU, "mu"),
                                                 (psums[1], blv_sb, LV, "lv")):
                    d = wp.tile([P, 1], F32, tag=f"d{tg}", name="d")
                    nc.vector.tensor_scalar_add(d[:], pd[:], EPS)
                    r = wp.tile([P, 1], F32, tag=f"r{tg}", name="r")
                    nc.vector.reciprocal(r[:], d[:])
                    o = wp.tile([P, out], F32, tag=f"o{tg}", name="o")
                    nc.vector.tensor_scalar(out=o[:], in0=pn[:], scalar1=r[:],
                                            scalar2=None, op0=ALU.mult)
                    nc.vector.tensor_tensor(out=o[:], in0=o[:], in1=bias[:], op=ALU.add)
                    nc.sync.dma_start(dest[ms, :], o[:])

            _edge_phase(nc, tc, cfg, pr, (cp, gp, wp, pp),
                        (XLchunks, XR, GIDX, RIDX, DCOL),
                        (iotaRep_sb, attB_sb, ones_sb), 2, finalize, dt)
    nc.compile()
    return nc


# ----------------------------------------------------------------------------
# Host orchestration
# ----------------------------------------------------------------------------

def _bb(v, rows=P):
    """Broadcast a 1-D row vector to [rows, len] f32."""
    v = np.asarray(v, np.float32).reshape(1, -1)
    return np.ascontiguousarray(np.broadcast_to(v, (rows, v.shape[1])))


def _hw_runner(nc, in_maps, cfg, trace=False):
    from concourse import bass_utils
    r = bass_utils.run_bass_kernel_spmd(
        nc, in_maps, core_ids=list(range(cfg.ncores)), trace=trace)
    return r.results, r.exec_time_ns


class _State:
    """Cached compiled programs + prep, keyed by edge structure."""
    key = None
    progs = None
    prep = None


EDT = BF16 if not int(os.environ.get("GAT_F32", "0")) else F32
NQUEUES = 4


def build_progs(cfg, pr, dt=None, nq=None):
    dt = EDT if dt is None else dt
    nq = NQUEUES if nq is None else nq
    return dict(
        dense1=build_dense1(cfg, dt),
        edge1=build_edge1(cfg, pr, dt, nq),
        edge23=build_edge23(cfg, pr, dt, nq),
    )


def forward(cfg, x, ei_unused, w, pr, progs, runner, dt=None):
    dt = EDT if dt is None else dt
    ndt = mybir.dt.np(dt)
    perm = pr["perm"]                    # [ncores, nlp] node ids or -1
    Tmax = int(pr["tbin"].max())
    profile = {}
    dcol_in = [np.ascontiguousarray(pr["dstcol"][c][:, :, None].astype(ndt))
               for c in range(cfg.ncores)]

    hid, out, nlp, ntab = cfg.hid, cfg.out, cfg.nlp, cfg.ntab

    # ---- launch A: dense1 --------------------------------------------------
    in_maps = []
    for c in range(cfg.ncores):
        xs = np.zeros((nlp, cfg.fin), np.float32)
        sel = perm[c] >= 0
        xs[sel] = x[perm[c][sel]]
        in_maps.append(dict(
            xT=np.ascontiguousarray(xs.T), wl=w["sh_Wl"], wr=w["sh_Wr"],
            blB=_bb(w["sh_bl"]), brB=_bb(w["sh_br"])))
    rA, profile["A"] = runner(progs["dense1"], in_maps, cfg)
    XL1full = np.concatenate([rA[c]["XL1"] for c in range(cfg.ncores)])
    XL1ch = {f"XL1c{o}": np.ascontiguousarray(
        XL1full[o * cfg.chrows:(o + 1) * cfg.chrows])
        for o in range(cfg.nchunk)}
    XR1 = [rA[c]["XR1"] for c in range(cfg.ncores)]

    # ---- launch B: edge1 + dense23 ----------------------------------------
    # iota along free dim, repeated Tmax times, same on every partition
    iotaRep = np.ascontiguousarray(np.broadcast_to(
        np.arange(P, dtype=np.float32), (P, Tmax, P))).astype(ndt)
    att1B = np.ascontiguousarray(np.broadcast_to(
        w["sh_att"].reshape(-1).astype(np.float32), (P, Tmax, hid))).astype(ndt)
    w23l = np.concatenate([w["mu_Wl"], w["lv_Wl"]], axis=1)
    w23r = np.concatenate([w["mu_Wr"], w["lv_Wr"]], axis=1)
    b23l = np.concatenate([w["mu_bl"], w["lv_bl"]])
    b23r = np.concatenate([w["mu_br"], w["lv_br"]])
    ident = np.eye(P, dtype=np.float32)
    in_maps = []
    for c in range(cfg.ncores):
        in_maps.append(dict(
            XR1=XR1[c], **XL1ch,
            gidx=pr["gidx16"][c], ridx=pr["ridx16"][c], dcol=dcol_in[c],
            iotaRep=iotaRep, attB=att1B, b1B=_bb(w["sh_b"]), identB=ident,
            w23l=w23l, w23r=w23r, b23lB=_bb(b23l), b23rB=_bb(b23r)))
    rB, profile["B"] = runner(progs["edge1"], in_maps, cfg)
    XL23full = np.concatenate([rB[c]["XL23"] for c in range(cfg.ncores)])
    XL23ch = {f"XL23c{o}": np.ascontiguousarray(
        XL23full[o * cfg.chrows:(o + 1) * cfg.chrows])
        for o in range(cfg.nchunk)}
    XR23 = [rB[c]["XR23"] for c in range(cfg.ncores)]

    # ---- launch C: edge23 --------------------------------------------------
    att23 = np.concatenate([w["mu_att"].reshape(-1),
                            w["lv_att"].reshape(-1)]).astype(np.float32)
    att23B = np.ascontiguousarray(np.broadcast_to(att23, (P, Tmax, P))).astype(ndt)
    in_maps = []
    for c in range(cfg.ncores):
        in_maps.append(dict(
            XR23=XR23[c], **XL23ch,
            gidx=pr["gidx16"][c], ridx=pr["ridx16"][c], dcol=dcol_in[c],
            iotaRep=iotaRep, attB=att23B,
            bmuB=_bb(w["mu_b"]), blvB=_bb(w["lv_b"])))
    rC, profile["C"] = runner(progs["edge23"], in_maps, cfg)

    MU = np.concatenate([rC[c]["MU"] for c in range(cfg.ncores)])
    LV = np.concatenate([rC[c]["LV"] for c in range(cfg.ncores)])
    mu = MU[pr["slot_global"]]
    lv = LV[pr["slot_global"]]
    return (mu, lv), profile


def kernel(**inputs):
    cfg = Cfg()
    x = np.asarray(inputs["x"], np.float32)
    ei = np.asarray(inputs["edge_index"]).astype(np.int64)
    w = {k: np.asarray(v, np.float32) for k, v in inputs.items()
         if k not in ("x", "edge_index")}

    key = hash(ei.tobytes())
    if _State.key != key:
        pr = prep_graph(cfg, ei)
        _State.prep = pr
        _State.progs = build_progs(cfg, pr)
        _State.key = key

    trace = bool(int(os.environ.get("GAT_TRACE", "0")))
    runner = functools.partial(_hw_runner, trace=trace)
    (mu, lv), profile = forward(cfg, x, ei, w, _State.prep, _State.progs, runner)
    kernel._last_profile = profile
    return (mu, lv)


kernel._last_profile = None

